# revision 21
# baseline (speedup 1.0000x reference)
import os
import sys

for _p in ("/opt/trn_rl_repo",):
    if os.path.isdir(_p) and _p not in sys.path:
        sys.path.insert(0, _p)

import numpy as np
import ml_dtypes
from concourse import bacc, tile, bass_utils
import concourse.bass as bass
from concourse.masks import make_identity

mybir = bass.mybir
dt = mybir.dt
Alu = mybir.AluOpType
Act = mybir.ActivationFunctionType
PM = mybir.MatmulPerfMode

B, S, D, L, FF = 16, 512, 512, 5, 1024
EPS = 1e-5
NCORES = 8
BPC = B // NCORES           # batches per core = 2
R = BPC * S                 # rows per core = 1024
NT = R // 128               # 8 row tiles
DC = D // 128               # 4 d chunks
DP = DC // 2                # 2 d chunk-pairs (fp8 DoubleRow)
FC = FF // 128              # 8 ff chunks
SC2 = 1.0 / float(D)        # the reference's double 1/sqrt(dk) scaling

# fp8 scales (powers of two)
SM = 4096.0                 # fused Wq@Wk^T fp8 scale
SH = 16.0                   # normalized-h fp8 scale
ST = 128.0                  # t1 = h @ M fp8 scale
SW = 1024.0                 # W1/W2 fp8 scale (fp8-FF layers)
SRELU = 32.0                # relu fp8 scale
KF = SRELU * SW             # 2^15: ff2 psum scale on fp8-FF layers
FP = FC // 2                # 4 ff chunk-pairs
FF8_LAYERS = (2,)         # layers whose FF runs in fp8 DoubleRow

F32 = dt.float32
BF16 = dt.bfloat16
FP8 = dt.float8e4

LAST_EXEC_NS = None
_CACHE = {}


def _build_program():
    nc = bacc.Bacc("TRN2", target_bir_lowering=False, debug=False,
                   num_devices=NCORES)

    h0_d = nc.dram_tensor("h0", [R, D], F32, kind="ExternalInput").ap()
    h0b_d = nc.dram_tensor("h0b", [R, D], BF16, kind="ExternalInput").ap()
    m8_d = nc.dram_tensor("m8", [L, DP, 128, 2, D], FP8,
                          kind="ExternalInput").ap()
    v16_d = nc.dram_tensor("v16", [L, D, D], BF16, kind="ExternalInput").ap()
    w1_d = nc.dram_tensor("w1", [L, D, FF], BF16, kind="ExternalInput").ap()
    w2_d = nc.dram_tensor("w2", [L, FF, D], BF16, kind="ExternalInput").ap()
    w18_d = nc.dram_tensor("w18", [L, DP, 128, 2, FF], FP8,
                           kind="ExternalInput").ap()
    w28_d = nc.dram_tensor("w28", [L, FP, 128, 2, D], FP8,
                           kind="ExternalInput").ap()
    cmT_d = nc.dram_tensor("cmT", [128, S], BF16, kind="ExternalInput").ap()
    out_d = nc.dram_tensor("out", [R, D], BF16, kind="ExternalOutput").ap()
    dma = nc.sync.dma_start

    with tile.TileContext(nc) as tc:
        with tc.tile_pool(name="sb", bufs=1) as sb, \
             tc.tile_pool(name="cst", bufs=1) as cst, \
             tc.tile_pool(name="ps", bufs=1, space="PSUM") as ps:

            # ---- constants ----
            ident = cst.tile([128, 128], F32, name="ident")
            make_identity(nc, ident)
            identB = cst.tile([128, 128], BF16, name="identB")
            nc.scalar.copy(identB[:], ident[:])
            ones = cst.tile([128, 1], BF16, name="ones")
            nc.gpsimd.memset(ones[:], 1.0)
            cmT = cst.tile([128, S], BF16, name="cmT")

            # ---- initial h (fp32 residual stream + bf16 matmul copy) ----
            h = []        # fp32 residual (scale-tracked, per-row affine ok)
            hbf = []      # bf16 normalized copies ([i, d]; also ph lhsT)
            for rt in range(NT):
                tb = sb.tile([128, D], BF16, tag="hbf", bufs=10,
                             name=f"h0b_{rt}")
                dma(tb[:], h0b_d[128 * rt:128 * (rt + 1), :])
                hbf.append(tb)

            # rstd of the "previous LN2" per batch ([128,4]); None for l=0
            rstd2 = [None] * BPC

            def transpose_fp8(src, b, lbl):
                """transpose batch b of bf16 [i,d] tiles -> fp8 [d,i] pair
                tiles [128, 2, 512] scaled by SH."""
                out = [None] * DP
                for p in range(DP):
                    out[p] = sb.tile([128, 2, S], FP8, tag="hT8", bufs=8,
                                     name=f"{lbl}{b}_{p}")
                for din in range(DC):
                    pt = ps.tile([128, S], BF16, tag="tr", bufs=2, name="trp")
                    for k in range(4):
                        nc.tensor.matmul(
                            pt[:, 128 * k:128 * (k + 1)],
                            src[4 * b + k][:, 128 * din:128 * (din + 1)],
                            identB[:],
                            is_transpose=True, start=True, stop=True,
                            skip_group_check=True)
                    if din % 2 == 0:
                        nc.vector.tensor_scalar(
                            out[din // 2][:, din % 2, :], pt[:], SH, None,
                            Alu.mult)
                    else:
                        nc.scalar.mul(out[din // 2][:, din % 2, :], pt[:], SH)
                return out

            def transpose_bf(src, b, lbl):
                """transpose batch b of bf16 [i,d] tiles -> bf16 [d,i]."""
                out = [None] * DC
                for din in range(DC):
                    pt = ps.tile([128, S], BF16, tag="tr", bufs=2, name="trp")
                    for k in range(4):
                        nc.tensor.matmul(
                            pt[:, 128 * k:128 * (k + 1)],
                            src[4 * b + k][:, 128 * din:128 * (din + 1)],
                            identB[:],
                            is_transpose=True, start=True, stop=True,
                            skip_group_check=True)
                    d_ = sb.tile([128, S], BF16, tag="h1T", bufs=8,
                                 name=f"{lbl}{b}_{din}")
                    nc.vector.tensor_copy(d_[:], pt[:])
                    out[din] = d_
                return out

            for l in range(L):
                # ---- layer weights (double-buffered across layers) ----
                m8 = [sb.tile([128, 2, D], FP8, tag="m8", bufs=4, name="m8")
                      for _ in range(DP)]
                for p in range(DP):
                    dma(m8[p][:], m8_d[l, p])
                if l == 0:
                    dma(cmT[:], cmT_d)
                v16 = sb.tile([128, DC * D], BF16, tag="v16", bufs=2,
                              name="v16")
                for c in range(DC):
                    dma(v16[:, D * c:D * (c + 1)],
                        v16_d[l, 128 * c:128 * (c + 1), :])
                ff8 = l in FF8_LAYERS
                if ff8:
                    w18 = [sb.tile([128, 2, FF], FP8, tag="w1", bufs=2,
                                   name="w18") for _ in range(DP)]
                    for p in range(DP):
                        dma(w18[p][:], w18_d[l, p])
                    w28 = [sb.tile([128, 2, D], FP8, tag="w28", bufs=8,
                                   name="w28") for _ in range(FP)]
                    for p in range(FP):
                        dma(w28[p][:], w28_d[l, p])
                else:
                    w1 = sb.tile([128, DC * FF], BF16, tag="w1", bufs=2,
                                 name="w1")
                    for c in range(DC):
                        dma(w1[:, FF * c:FF * (c + 1)],
                            w1_d[l, 128 * c:128 * (c + 1), :])
                    w2 = sb.tile([128, FC * D], BF16, tag="w2", bufs=2,
                                 name="w2")
                    for c in range(FC):
                        dma(w2[:, D * c:D * (c + 1)],
                            w2_d[l, 128 * c:128 * (c + 1), :])
                if l == 0:
                    # h fp32 (residual stream) is first read at the out-proj
                    # residual add; DMA configs go after the layer-0 weights
                    for rt in range(NT):
                        t_ = sb.tile([128, D], F32, tag="h", bufs=16,
                                     name=f"h0_{rt}")
                        dma(t_[:], h0_d[128 * rt:128 * (rt + 1), :])
                        h.append(t_)

                s1 = sb.tile([128, NT], F32, tag="st8", bufs=8, name="s1")
                s2 = sb.tile([128, NT], F32, tag="st8", bufs=8, name="s2")
                f1 = sb.tile([128, NT], F32, tag="st8", bufs=8, name="f1")
                f2 = sb.tile([128, NT], F32, tag="st8", bufs=8, name="f2")
                esum = sb.tile([128, NT], F32, tag="st8", bufs=8, name="esum")

                # ---- transpose h; t1 = h @ (WqWk^T) per batch ----
                hT8 = [None] * BPC
                t18 = [[None] * DP for _ in range(BPC)]
                for b in range(BPC):
                    hT8[b] = transpose_fp8(hbf, b, f"h{l}T")
                    for m in range(DC):
                        pq = ps.tile([128, S], F32, tag="mm", bufs=3,
                                     name="pq")
                        msl = slice(128 * m, 128 * (m + 1))
                        for p in range(DP):
                            nc.tensor.matmul(pq[:], m8[p][:, :, msl],
                                             hT8[b][p][:],
                                             start=(p == 0), stop=(p == 1),
                                             perf_mode=PM.DoubleRow)
                        if t18[b][m // 2] is None:
                            t18[b][m // 2] = sb.tile([128, 2, S], FP8,
                                                     tag="t18", bufs=8,
                                                     name="t18")
                        if m % 2 == 0:
                            nc.scalar.mul(t18[b][m // 2][:, m % 2, :], pq[:],
                                          ST / (SH * SM))
                        else:
                            nc.vector.tensor_scalar(
                                t18[b][m // 2][:, m % 2, :], pq[:],
                                ST / (SH * SM), None, Alu.mult)

                # ---- scores^T [j, i] (triangular) + exp + esum ----
                pexpT = [[None] * 4 for _ in range(BPC)]
                for b in range(BPC):
                    for jc in range(4):
                        w = S - 128 * jc
                        sl = slice(128 * jc, S)
                        jsl = slice(128 * jc, 128 * (jc + 1))
                        sc = ps.tile([128, S], F32, tag="sc", bufs=2,
                                     name="sc")
                        nc.tensor.matmul(sc[:, sl], hT8[b][0][:, :, jsl],
                                         t18[b][0][:, :, sl],
                                         start=True, stop=False,
                                         perf_mode=PM.DoubleRow)
                        nc.tensor.matmul(sc[:, sl], identB[:],
                                         cmT[:, 0:w],
                                         start=False, stop=False,
                                         skip_group_check=True)
                        nc.tensor.matmul(sc[:, sl], hT8[b][1][:, :, jsl],
                                         t18[b][1][:, :, sl],
                                         start=False, stop=True,
                                         perf_mode=PM.DoubleRow)
                        pe_ = sb.tile([128, S], BF16, tag="p", bufs=8,
                                      name="pexpT")
                        nc.scalar.activation(pe_[:, sl], sc[:, sl], Act.Exp,
                                             scale=SC2 / (ST * SH))
                        pexpT[b][jc] = pe_
                    # esum[i] = sum_j exp: ones-matmul, triangular accumulate
                    pes = ps.tile([1, S], F32, tag="es", bufs=1, name="pes")
                    for jc in range(4):
                        sl = slice(128 * jc, S)
                        nc.tensor.matmul(pes[:, sl], ones[:],
                                         pexpT[b][jc][:, sl],
                                         start=(jc == 0), stop=(jc == 3),
                                         skip_group_check=True)
                    # redistribute [1, 512] -> [128, 4] (col t = row block)
                    essb = sb.tile([1, S], F32, tag="essb", bufs=2,
                                   name="essb")
                    nc.scalar.copy(essb[:], pes[:])
                    for t in range(4):
                        dma(esum[:, 4 * b + t:4 * b + t + 1],
                            essb[:, 128 * t:128 * (t + 1)])


                def ln_stats(sa, sb_, b, rk=None, vsc=1.0, rsc=1.0):
                    """LN stats from accumulated sum/sumsq [128, 4b:4b+4].
                    rk: per-row 1/c factor (applied squared to var, then
                    multiplied into rstd). Returns (mu, rstd) in the STORED
                    residual scale."""
                    csl = slice(4 * b, 4 * b + 4)
                    mu = sb.tile([128, 4], F32, tag="st4", bufs=16, name="mu")
                    nc.vector.tensor_scalar(mu[:], sa[:, csl], 1.0 / D, None,
                                            Alu.mult)
                    musq = sb.tile([128, 4], F32, tag="st4", bufs=16,
                                   name="musq")
                    nc.vector.tensor_tensor(musq[:], mu[:], mu[:], Alu.mult)
                    var = sb.tile([128, 4], F32, tag="st4", bufs=16,
                                  name="var")
                    nc.vector.scalar_tensor_tensor(
                        var[:], sb_[:, csl], 1.0 / D, musq[:],
                        Alu.mult, Alu.subtract)
                    if rk is not None:
                        rec2 = sb.tile([128, 4], F32, tag="st4", bufs=16,
                                       name="rec2")
                        nc.vector.tensor_tensor(rec2[:], rk[:], rk[:],
                                                Alu.mult)
                        nc.vector.tensor_tensor(var[:], var[:], rec2[:],
                                                Alu.mult)
                    ve = sb.tile([128, 4], F32, tag="st4", bufs=16, name="ve")
                    nc.vector.tensor_scalar(ve[:], var[:], vsc, vsc * EPS,
                                            Alu.mult, Alu.add)
                    y = sb.tile([128, 4], F32, tag="st4", bufs=16, name="y")
                    nc.vector.tensor_scalar(y[:], ve[:], -0.5, 1.5,
                                            Alu.mult, Alu.add)
                    t1_ = sb.tile([128, 4], F32, tag="st4", bufs=16,
                                  name="t1_")
                    nc.vector.tensor_tensor(t1_[:], y[:], y[:], Alu.mult)
                    nc.vector.tensor_tensor(t1_[:], t1_[:], ve[:], Alu.mult)
                    nc.vector.tensor_scalar(t1_[:], t1_[:], -0.5, 1.5,
                                            Alu.mult, Alu.add)
                    rstd = sb.tile([128, 4], F32, tag="st4", bufs=16,
                                   name="rstd")
                    nc.vector.tensor_tensor(rstd[:], y[:], t1_[:], Alu.mult)
                    if rk is not None:
                        nc.vector.scalar_tensor_tensor(
                            rstd[:], rstd[:], rsc, rk[:], Alu.mult, Alu.mult)
                    elif rsc != 1.0:
                        nc.vector.tensor_scalar(rstd[:], rstd[:], rsc, None,
                                                Alu.mult)
                    return mu, rstd

                # ---- per batch: ph^T = h^T p^T; x_att; residual; LN1 ----
                hres = [None] * NT
                h1bf = [None] * NT
                rstd1 = [None] * BPC
                for b in range(BPC):
                    csl = slice(4 * b, 4 * b + 4)
                    rec = sb.tile([128, 4], F32, tag="st4", bufs=16,
                                  name="rec")
                    nc.vector.reciprocal(rec[:], esum[:, csl])
                    sA = None
                    if rstd2[b] is not None:
                        sA = sb.tile([128, 4], F32, tag="st4", bufs=16,
                                     name="sA")
                        nc.vector.tensor_tensor(sA[:], esum[:, csl],
                                                rstd2[b][:], Alu.mult)
                    phT = [None] * DC
                    for dtile in range(DC):
                        pc = ps.tile([128, S], F32, tag="mm", bufs=3,
                                     name="pc")
                        dsl = slice(128 * dtile, 128 * (dtile + 1))
                        for jc in range(4):
                            nc.tensor.matmul(
                                pc[:, 128 * jc:S],
                                hbf[4 * b + jc][:, dsl],
                                pexpT[b][jc][:, 128 * jc:S],
                                start=(jc == 0), stop=(jc == 3),
                                skip_group_check=True)
                        t_ = sb.tile([128, S], BF16, tag="phT", bufs=8,
                                     name="phT")
                        if dtile % 2 == 0:
                            nc.scalar.copy(t_[:], pc[:])
                        else:
                            nc.vector.tensor_copy(t_[:], pc[:])
                        phT[dtile] = t_
                    for t in range(4):
                        rt = 4 * b + t
                        px = ps.tile([128, D], F32, tag="mm", bufs=3,
                                     name="px")
                        for dc_ in range(DC):
                            nc.tensor.matmul(
                                px[:],
                                phT[dc_][:, 128 * t:128 * (t + 1)],
                                v16[:, D * dc_:D * (dc_ + 1)],
                                start=(dc_ == 0), stop=(dc_ == DC - 1))
                        res = sb.tile([128, D], F32, tag="h", bufs=16,
                                      name="res")
                        sA_t = (esum[:, rt:rt + 1] if sA is None
                                else sA[:, t:t + 1])
                        nc.vector.scalar_tensor_tensor(
                            res[:], h[rt][:], sA_t, px[:],
                            Alu.mult, Alu.add, accum_out=s1[:, rt:rt + 1])
                        scr = sb.tile([128, D], F32, tag="scr", bufs=2,
                                      name="scr")
                        nc.scalar.activation(scr[:], res[:], Act.Square,
                                             accum_out=s2[:, rt:rt + 1])
                        hres[rt] = res
                    recs_b = rec
                    if l == 0:
                        mu1, r1 = ln_stats(s1, s2, b, rk=recs_b,
                                           vsc=2048.0, rsc=45.254834)
                    else:
                        mu1, r1 = ln_stats(s1, s2, b, rk=recs_b)
                    rstd1[b] = r1
                    nm1 = sb.tile([128, 4], F32, tag="st4", bufs=16,
                                  name="nm1")
                    nc.vector.scalar_tensor_tensor(
                        nm1[:], mu1[:], -1.0, r1[:], Alu.mult, Alu.mult)
                    for t in range(4):
                        rt = 4 * b + t
                        tb = sb.tile([128, D], BF16, tag="h1bf", bufs=8,
                                     name=f"h1bf{rt}")
                        if t % 2 == 0:
                            nc.scalar.activation(tb[:], hres[rt][:],
                                                 Act.Identity,
                                                 bias=nm1[:, t:t + 1],
                                                 scale=r1[:, t:t + 1])
                        else:
                            nc.vector.tensor_scalar(
                                tb[:], hres[rt][:],
                                mu1[:, t:t + 1], r1[:, t:t + 1],
                                Alu.subtract, Alu.mult)
                        h1bf[rt] = tb

                # ---- feed-forward + residual + LN2 ----
                h_next = [None] * NT
                hbf_next = [None] * NT
                for b in range(BPC):
                    if ff8:
                        h1T8 = transpose_fp8(h1bf, b, f"g{l}T")
                        relu8 = [sb.tile([128, 2, S], FP8, tag="relu",
                                         bufs=16, name="relu8")
                                 for _ in range(FP)]
                        for f in range(FC):
                            pf = ps.tile([128, S], F32, tag="mm", bufs=3,
                                         name="pf")
                            fsl = slice(128 * f, 128 * (f + 1))
                            for p in range(DP):
                                nc.tensor.matmul(pf[:], w18[p][:, :, fsl],
                                                 h1T8[p][:],
                                                 start=(p == 0),
                                                 stop=(p == 1),
                                                 perf_mode=PM.DoubleRow)
                            dst = relu8[f // 2][:, f % 2, :]
                            if f % 2 == 0:
                                nc.scalar.activation(dst, pf[:], Act.Relu,
                                                     scale=SRELU / (SH * SW))
                            else:
                                nc.vector.tensor_scalar(
                                    dst, pf[:], SRELU / (SH * SW), 0.0,
                                    Alu.mult, Alu.max)
                        sF = sb.tile([128, 4], F32, tag="st4", bufs=16,
                                     name="sF")
                        nc.vector.tensor_scalar(sF[:], rstd1[b][:], KF, None,
                                                Alu.mult)
                    else:
                        h1T = transpose_bf(h1bf, b, f"g{l}T")
                        relu = []
                        for f in range(FC):
                            pf = ps.tile([128, S], F32, tag="mm", bufs=3,
                                         name="pf")
                            for din in range(DC):
                                nc.tensor.matmul(
                                    pf[:],
                                    w1[:, FF * din + 128 * f:
                                       FF * din + 128 * (f + 1)],
                                    h1T[din][:],
                                    start=(din == 0), stop=(din == DC - 1))
                            tr_ = sb.tile([128, S], BF16, tag="relu", bufs=16,
                                          name="relu")
                            if f % 2 == 0:
                                nc.scalar.activation(tr_[:], pf[:], Act.Relu)
                            else:
                                nc.vector.tensor_relu(tr_[:], pf[:])
                            relu.append(tr_)
                    for t in range(4):
                        rt = 4 * b + t
                        pd = ps.tile([128, D], F32, tag="mm", bufs=3,
                                     name="pd")
                        if ff8:
                            for p in range(FP):
                                nc.tensor.matmul(
                                    pd[:],
                                    relu8[p][:, :, 128 * t:128 * (t + 1)],
                                    w28[p][:],
                                    start=(p == 0), stop=(p == FP - 1),
                                    perf_mode=PM.DoubleRow)
                            sF_t = sF[:, t:t + 1]
                        else:
                            for fc in range(FC):
                                nc.tensor.matmul(
                                    pd[:],
                                    relu[fc][:, 128 * t:128 * (t + 1)],
                                    w2[:, D * fc:D * (fc + 1)],
                                    start=(fc == 0), stop=(fc == FC - 1))
                            sF_t = rstd1[b][:, t:t + 1]
                        res2 = sb.tile([128, D], F32, tag="h", bufs=16,
                                       name="res2")
                        nc.vector.scalar_tensor_tensor(
                            res2[:], hres[rt][:], sF_t, pd[:],
                            Alu.mult, Alu.add, accum_out=f1[:, rt:rt + 1])
                        scr2 = sb.tile([128, D], F32, tag="scr", bufs=2,
                                       name="scr2")
                        nc.scalar.activation(scr2[:], res2[:], Act.Square,
                                             accum_out=f2[:, rt:rt + 1])
                        h_next[rt] = res2
                    if ff8:
                        mu2, r2 = ln_stats(f1, f2, b, vsc=1.0 / (KF * KF),
                                           rsc=1.0 / KF)
                    else:
                        mu2, r2 = ln_stats(f1, f2, b)
                    rstd2[b] = r2
                    nm2 = sb.tile([128, 4], F32, tag="st4", bufs=16,
                                  name="nm2")
                    nc.vector.scalar_tensor_tensor(
                        nm2[:], mu2[:], -1.0, r2[:], Alu.mult, Alu.mult)
                    for t in range(4):
                        rt = 4 * b + t
                        tb = sb.tile([128, D], BF16, tag="hbf", bufs=10,
                                     name=f"nbf{rt}")
                        if t % 2 == 0:
                            nc.scalar.activation(tb[:], h_next[rt][:],
                                                 Act.Identity,
                                                 bias=nm2[:, t:t + 1],
                                                 scale=r2[:, t:t + 1])
                        else:
                            nc.vector.tensor_scalar(tb[:], h_next[rt][:],
                                                    mu2[:, t:t + 1],
                                                    r2[:, t:t + 1],
                                                    Alu.subtract, Alu.mult)
                        hbf_next[rt] = tb

                h = h_next
                hbf = hbf_next

            for rt in range(NT):
                dma(out_d[128 * rt:128 * (rt + 1), :], hbf[rt][:])

    nc.compile()
    return nc


def _host_inputs(inputs):
    x = np.asarray(inputs["x"])
    tok_emb = np.asarray(inputs["tok_emb"], dtype=np.float32)

    for nm in ("bq", "bk", "bv", "bo", "b1", "b2", "ln1_b", "ln2_b"):
        assert np.allclose(np.asarray(inputs[nm]), 0.0), f"{nm} nonzero"
    for nm in ("ln1_g", "ln2_g"):
        assert np.allclose(np.asarray(inputs[nm]), 1.0), f"{nm} != 1"

    f8 = ml_dtypes.float8_e4m3fn
    bf = ml_dtypes.bfloat16

    wq = np.asarray(inputs["Wq"], np.float64)
    wk = np.asarray(inputs["Wk"], np.float64)
    wv = np.asarray(inputs["Wv"], np.float64)
    wo = np.asarray(inputs["Wo"], np.float64)
    m = np.einsum('lde,lfe->ldf', wq, wk).astype(np.float32)  # Wq @ Wk^T
    v = np.einsum('lde,lef->ldf', wv, wo).astype(np.float32)  # Wv @ Wo

    m = m * SM
    assert np.abs(m).max() < 240.0, "fp8 overflow in fused M"

    def pairs8(w, scale):
        # [L, K, n] -> [L, K/256, 128, 2, n] with row k = 128*(2p+e)+r
        w = np.asarray(w, np.float32) * scale
        assert np.abs(w).max() < 240.0, "fp8 overflow"
        Lx, K, n = w.shape
        w = w.reshape(Lx, K // 256, 2, 128, n).transpose(0, 1, 3, 2, 4)
        return np.ascontiguousarray(w).astype(f8)

    # pair layout [L, DP, 128, 2, D]: row k = 128*(2p+e)+r
    m = m.reshape(L, DP, 2, 128, D).transpose(0, 1, 3, 2, 4)
    shared = {
        "m8": np.ascontiguousarray(m).astype(f8),
        "v16": v.astype(bf),
        "w1": np.asarray(inputs["W1"], np.float32).astype(bf),
        "w2": np.asarray(inputs["W2"], np.float32).astype(bf),
        "w18": pairs8(inputs["W1"], SW),
        "w28": pairs8(inputs["W2"], SW),
    }
    # transposed causal mask, diagonal block then zero padding:
    # col 0..127 = (0 if i >= j else -1e9), cols 128.. = 0
    jj = np.arange(128)
    cmT = np.zeros((128, S), dtype=np.float32)
    cmT[:, :128] = np.where(jj[None, :] >= jj[:, None], 0.0, -1e9)
    shared["cmT"] = cmT.astype(bf)

    h0 = tok_emb[x.astype(np.int64)]  # [B, S, D] fp32
    return shared, h0


def kernel(**inputs):
    global LAST_EXEC_NS
    shared, h0 = _host_inputs(inputs)

    if "prog" not in _CACHE:
        _CACHE["prog"] = _build_program()
    nc = _CACHE["prog"]

    in_maps = []
    for c in range(NCORES):
        m = dict(shared)
        m["h0"] = np.ascontiguousarray(
            h0[BPC * c:BPC * (c + 1)].reshape(R, D))
        m["h0b"] = m["h0"].astype(ml_dtypes.bfloat16)
        in_maps.append(m)

    trace = bool(int(os.environ.get("KERNEL_TRACE", "0")))
    res = bass_utils.run_bass_kernel_spmd(
        nc, in_maps, core_ids=list(range(NCORES)), trace=trace)
    LAST_EXEC_NS = res.exec_time_ns

    out = np.concatenate(
        [res.results[c]["out"].reshape(BPC, S, D) for c in range(NCORES)],
        axis=0)
    return out.astype(np.float32)


# revision 22
# speedup vs baseline: 1.0184x; 1.0184x over previous
import os
import sys

for _p in ("/opt/trn_rl_repo",):
    if os.path.isdir(_p) and _p not in sys.path:
        sys.path.insert(0, _p)

import numpy as np
import ml_dtypes
from concourse import bacc, tile, bass_utils
import concourse.bass as bass
from concourse.masks import make_identity

mybir = bass.mybir
dt = mybir.dt
Alu = mybir.AluOpType
Act = mybir.ActivationFunctionType
PM = mybir.MatmulPerfMode

B, S, D, L, FF = 16, 512, 512, 5, 1024
EPS = 1e-5
NCORES = 8
BPC = B // NCORES           # batches per core = 2
R = BPC * S                 # rows per core = 1024
NT = R // 128               # 8 row tiles
DC = D // 128               # 4 d chunks
DP = DC // 2                # 2 d chunk-pairs (fp8 DoubleRow)
FC = FF // 128              # 8 ff chunks
SC2 = 1.0 / float(D)        # the reference's double 1/sqrt(dk) scaling

# fp8 scales (powers of two)
SM = 4096.0                 # fused Wq@Wk^T fp8 scale
SH = 16.0                   # normalized-h fp8 scale
ST = 128.0                  # t1 = h @ M fp8 scale
SW = 1024.0                 # W1/W2 fp8 scale (fp8-FF layers)
SRELU = 32.0                # relu fp8 scale
KF = SRELU * SW             # 2^15: ff2 psum scale on fp8-FF layers
FP = FC // 2                # 4 ff chunk-pairs
FF8_LAYERS = (2, 3)       # layers whose FF runs in fp8 DoubleRow

F32 = dt.float32
BF16 = dt.bfloat16
FP8 = dt.float8e4

LAST_EXEC_NS = None
_CACHE = {}


def _build_program():
    nc = bacc.Bacc("TRN2", target_bir_lowering=False, debug=False,
                   num_devices=NCORES)

    h0_d = nc.dram_tensor("h0", [R, D], F32, kind="ExternalInput").ap()
    h0b_d = nc.dram_tensor("h0b", [R, D], BF16, kind="ExternalInput").ap()
    m8_d = nc.dram_tensor("m8", [L, DP, 128, 2, D], FP8,
                          kind="ExternalInput").ap()
    v16_d = nc.dram_tensor("v16", [L, D, D], BF16, kind="ExternalInput").ap()
    w1_d = nc.dram_tensor("w1", [L, D, FF], BF16, kind="ExternalInput").ap()
    w2_d = nc.dram_tensor("w2", [L, FF, D], BF16, kind="ExternalInput").ap()
    w18_d = nc.dram_tensor("w18", [L, DP, 128, 2, FF], FP8,
                           kind="ExternalInput").ap()
    w28_d = nc.dram_tensor("w28", [L, FP, 128, 2, D], FP8,
                           kind="ExternalInput").ap()
    cmT_d = nc.dram_tensor("cmT", [128, S], BF16, kind="ExternalInput").ap()
    out_d = nc.dram_tensor("out", [R, D], BF16, kind="ExternalOutput").ap()
    dma = nc.sync.dma_start

    with tile.TileContext(nc) as tc:
        with tc.tile_pool(name="sb", bufs=1) as sb, \
             tc.tile_pool(name="cst", bufs=1) as cst, \
             tc.tile_pool(name="ps", bufs=1, space="PSUM") as ps:

            # ---- constants ----
            ident = cst.tile([128, 128], F32, name="ident")
            make_identity(nc, ident)
            identB = cst.tile([128, 128], BF16, name="identB")
            nc.scalar.copy(identB[:], ident[:])
            ones = cst.tile([128, 1], BF16, name="ones")
            nc.gpsimd.memset(ones[:], 1.0)
            cmT = cst.tile([128, S], BF16, name="cmT")

            # ---- initial h (fp32 residual stream + bf16 matmul copy) ----
            h = []        # fp32 residual (scale-tracked, per-row affine ok)
            hbf = []      # bf16 normalized copies ([i, d]; also ph lhsT)
            for rt in range(NT):
                tb = sb.tile([128, D], BF16, tag="hbf", bufs=10,
                             name=f"h0b_{rt}")
                dma(tb[:], h0b_d[128 * rt:128 * (rt + 1), :])
                hbf.append(tb)

            # rstd of the "previous LN2" per batch ([128,4]); None for l=0
            rstd2 = [None] * BPC

            def transpose_fp8(src, b, lbl):
                """transpose batch b of bf16 [i,d] tiles -> fp8 [d,i] pair
                tiles [128, 2, 512] scaled by SH."""
                out = [None] * DP
                for p in range(DP):
                    out[p] = sb.tile([128, 2, S], FP8, tag="hT8", bufs=8,
                                     name=f"{lbl}{b}_{p}")
                for din in range(DC):
                    pt = ps.tile([128, S], BF16, tag="tr", bufs=2, name="trp")
                    for k in range(4):
                        nc.tensor.matmul(
                            pt[:, 128 * k:128 * (k + 1)],
                            src[4 * b + k][:, 128 * din:128 * (din + 1)],
                            identB[:],
                            is_transpose=True, start=True, stop=True,
                            skip_group_check=True)
                    if din % 2 == 0:
                        nc.vector.tensor_scalar(
                            out[din // 2][:, din % 2, :], pt[:], SH, None,
                            Alu.mult)
                    else:
                        nc.scalar.mul(out[din // 2][:, din % 2, :], pt[:], SH)
                return out

            def transpose_bf(src, b, lbl):
                """transpose batch b of bf16 [i,d] tiles -> bf16 [d,i]."""
                out = [None] * DC
                for din in range(DC):
                    pt = ps.tile([128, S], BF16, tag="tr", bufs=2, name="trp")
                    for k in range(4):
                        nc.tensor.matmul(
                            pt[:, 128 * k:128 * (k + 1)],
                            src[4 * b + k][:, 128 * din:128 * (din + 1)],
                            identB[:],
                            is_transpose=True, start=True, stop=True,
                            skip_group_check=True)
                    d_ = sb.tile([128, S], BF16, tag="h1T", bufs=8,
                                 name=f"{lbl}{b}_{din}")
                    nc.vector.tensor_copy(d_[:], pt[:])
                    out[din] = d_
                return out

            for l in range(L):
                # ---- layer weights (double-buffered across layers) ----
                m8 = [sb.tile([128, 2, D], FP8, tag="m8", bufs=4, name="m8")
                      for _ in range(DP)]
                for p in range(DP):
                    dma(m8[p][:], m8_d[l, p])
                if l == 0:
                    dma(cmT[:], cmT_d)
                v16 = sb.tile([128, DC * D], BF16, tag="v16", bufs=2,
                              name="v16")
                for c in range(DC):
                    dma(v16[:, D * c:D * (c + 1)],
                        v16_d[l, 128 * c:128 * (c + 1), :])
                ff8 = l in FF8_LAYERS
                if ff8:
                    w18 = [sb.tile([128, 2, FF], FP8, tag="w1", bufs=2,
                                   name="w18") for _ in range(DP)]
                    for p in range(DP):
                        dma(w18[p][:], w18_d[l, p])
                    w28 = [sb.tile([128, 2, D], FP8, tag="w28", bufs=8,
                                   name="w28") for _ in range(FP)]
                    for p in range(FP):
                        dma(w28[p][:], w28_d[l, p])
                else:
                    w1 = sb.tile([128, DC * FF], BF16, tag="w1", bufs=2,
                                 name="w1")
                    for c in range(DC):
                        dma(w1[:, FF * c:FF * (c + 1)],
                            w1_d[l, 128 * c:128 * (c + 1), :])
                    w2 = sb.tile([128, FC * D], BF16, tag="w2", bufs=2,
                                 name="w2")
                    for c in range(FC):
                        dma(w2[:, D * c:D * (c + 1)],
                            w2_d[l, 128 * c:128 * (c + 1), :])
                if l == 0:
                    # h fp32 (residual stream) is first read at the out-proj
                    # residual add; DMA configs go after the layer-0 weights
                    for rt in range(NT):
                        t_ = sb.tile([128, D], F32, tag="h", bufs=16,
                                     name=f"h0_{rt}")
                        dma(t_[:], h0_d[128 * rt:128 * (rt + 1), :])
                        h.append(t_)

                s1 = sb.tile([128, NT], F32, tag="st8", bufs=8, name="s1")
                s2 = sb.tile([128, NT], F32, tag="st8", bufs=8, name="s2")
                f1 = sb.tile([128, NT], F32, tag="st8", bufs=8, name="f1")
                f2 = sb.tile([128, NT], F32, tag="st8", bufs=8, name="f2")
                esum = sb.tile([128, NT], F32, tag="st8", bufs=8, name="esum")

                # ---- transpose h; t1 = h @ (WqWk^T) per batch ----
                hT8 = [None] * BPC
                t18 = [[None] * DP for _ in range(BPC)]
                for b in range(BPC):
                    hT8[b] = transpose_fp8(hbf, b, f"h{l}T")
                    for m in range(DC):
                        pq = ps.tile([128, S], F32, tag="mm", bufs=3,
                                     name="pq")
                        msl = slice(128 * m, 128 * (m + 1))
                        for p in range(DP):
                            nc.tensor.matmul(pq[:], m8[p][:, :, msl],
                                             hT8[b][p][:],
                                             start=(p == 0), stop=(p == 1),
                                             perf_mode=PM.DoubleRow)
                        if t18[b][m // 2] is None:
                            t18[b][m // 2] = sb.tile([128, 2, S], FP8,
                                                     tag="t18", bufs=8,
                                                     name="t18")
                        if m % 2 == 0:
                            nc.scalar.mul(t18[b][m // 2][:, m % 2, :], pq[:],
                                          ST / (SH * SM))
                        else:
                            nc.vector.tensor_scalar(
                                t18[b][m // 2][:, m % 2, :], pq[:],
                                ST / (SH * SM), None, Alu.mult)

                # ---- scores^T [j, i] (triangular) + exp + esum ----
                pexpT = [[None] * 4 for _ in range(BPC)]
                for b in range(BPC):
                    for jc in range(4):
                        w = S - 128 * jc
                        sl = slice(128 * jc, S)
                        jsl = slice(128 * jc, 128 * (jc + 1))
                        sc = ps.tile([128, S], F32, tag="sc", bufs=2,
                                     name="sc")
                        nc.tensor.matmul(sc[:, sl], hT8[b][0][:, :, jsl],
                                         t18[b][0][:, :, sl],
                                         start=True, stop=False,
                                         perf_mode=PM.DoubleRow)
                        nc.tensor.matmul(sc[:, sl], identB[:],
                                         cmT[:, 0:w],
                                         start=False, stop=False,
                                         skip_group_check=True)
                        nc.tensor.matmul(sc[:, sl], hT8[b][1][:, :, jsl],
                                         t18[b][1][:, :, sl],
                                         start=False, stop=True,
                                         perf_mode=PM.DoubleRow)
                        pe_ = sb.tile([128, S], BF16, tag="p", bufs=8,
                                      name="pexpT")
                        nc.scalar.activation(pe_[:, sl], sc[:, sl], Act.Exp,
                                             scale=SC2 / (ST * SH))
                        pexpT[b][jc] = pe_
                    # esum[i] = sum_j exp: ones-matmul, triangular accumulate
                    pes = ps.tile([1, S], F32, tag="es", bufs=1, name="pes")
                    for jc in range(4):
                        sl = slice(128 * jc, S)
                        nc.tensor.matmul(pes[:, sl], ones[:],
                                         pexpT[b][jc][:, sl],
                                         start=(jc == 0), stop=(jc == 3),
                                         skip_group_check=True)
                    # redistribute [1, 512] -> [128, 4] (col t = row block)
                    essb = sb.tile([1, S], F32, tag="essb", bufs=2,
                                   name="essb")
                    nc.scalar.copy(essb[:], pes[:])
                    for t in range(4):
                        dma(esum[:, 4 * b + t:4 * b + t + 1],
                            essb[:, 128 * t:128 * (t + 1)])


                def ln_stats(sa, sb_, b, rk=None, vsc=1.0, rsc=1.0):
                    """LN stats from accumulated sum/sumsq [128, 4b:4b+4].
                    rk: per-row 1/c factor (applied squared to var, then
                    multiplied into rstd). Returns (mu, rstd) in the STORED
                    residual scale."""
                    csl = slice(4 * b, 4 * b + 4)
                    mu = sb.tile([128, 4], F32, tag="st4", bufs=16, name="mu")
                    nc.vector.tensor_scalar(mu[:], sa[:, csl], 1.0 / D, None,
                                            Alu.mult)
                    musq = sb.tile([128, 4], F32, tag="st4", bufs=16,
                                   name="musq")
                    nc.vector.tensor_tensor(musq[:], mu[:], mu[:], Alu.mult)
                    var = sb.tile([128, 4], F32, tag="st4", bufs=16,
                                  name="var")
                    nc.vector.scalar_tensor_tensor(
                        var[:], sb_[:, csl], 1.0 / D, musq[:],
                        Alu.mult, Alu.subtract)
                    if rk is not None:
                        rec2 = sb.tile([128, 4], F32, tag="st4", bufs=16,
                                       name="rec2")
                        nc.vector.tensor_tensor(rec2[:], rk[:], rk[:],
                                                Alu.mult)
                        nc.vector.tensor_tensor(var[:], var[:], rec2[:],
                                                Alu.mult)
                    ve = sb.tile([128, 4], F32, tag="st4", bufs=16, name="ve")
                    nc.vector.tensor_scalar(ve[:], var[:], vsc, vsc * EPS,
                                            Alu.mult, Alu.add)
                    y = sb.tile([128, 4], F32, tag="st4", bufs=16, name="y")
                    nc.vector.tensor_scalar(y[:], ve[:], -0.5, 1.5,
                                            Alu.mult, Alu.add)
                    t1_ = sb.tile([128, 4], F32, tag="st4", bufs=16,
                                  name="t1_")
                    nc.vector.tensor_tensor(t1_[:], y[:], y[:], Alu.mult)
                    nc.vector.tensor_tensor(t1_[:], t1_[:], ve[:], Alu.mult)
                    nc.vector.tensor_scalar(t1_[:], t1_[:], -0.5, 1.5,
                                            Alu.mult, Alu.add)
                    rstd = sb.tile([128, 4], F32, tag="st4", bufs=16,
                                   name="rstd")
                    nc.vector.tensor_tensor(rstd[:], y[:], t1_[:], Alu.mult)
                    if rk is not None:
                        nc.vector.scalar_tensor_tensor(
                            rstd[:], rstd[:], rsc, rk[:], Alu.mult, Alu.mult)
                    elif rsc != 1.0:
                        nc.vector.tensor_scalar(rstd[:], rstd[:], rsc, None,
                                                Alu.mult)
                    return mu, rstd

                # ---- per batch: ph^T = h^T p^T; x_att; residual; LN1 ----
                hres = [None] * NT
                h1bf = [None] * NT
                rstd1 = [None] * BPC
                for b in range(BPC):
                    csl = slice(4 * b, 4 * b + 4)
                    rec = sb.tile([128, 4], F32, tag="st4", bufs=16,
                                  name="rec")
                    nc.vector.reciprocal(rec[:], esum[:, csl])
                    sA = None
                    if rstd2[b] is not None:
                        sA = sb.tile([128, 4], F32, tag="st4", bufs=16,
                                     name="sA")
                        nc.vector.tensor_tensor(sA[:], esum[:, csl],
                                                rstd2[b][:], Alu.mult)
                    phT = [None] * DC
                    for dtile in range(DC):
                        pc = ps.tile([128, S], F32, tag="mm", bufs=3,
                                     name="pc")
                        dsl = slice(128 * dtile, 128 * (dtile + 1))
                        for jc in range(4):
                            nc.tensor.matmul(
                                pc[:, 128 * jc:S],
                                hbf[4 * b + jc][:, dsl],
                                pexpT[b][jc][:, 128 * jc:S],
                                start=(jc == 0), stop=(jc == 3),
                                skip_group_check=True)
                        t_ = sb.tile([128, S], BF16, tag="phT", bufs=8,
                                     name="phT")
                        if dtile % 2 == 0:
                            nc.scalar.copy(t_[:], pc[:])
                        else:
                            nc.vector.tensor_copy(t_[:], pc[:])
                        phT[dtile] = t_
                    for t in range(4):
                        rt = 4 * b + t
                        px = ps.tile([128, D], F32, tag="mm", bufs=3,
                                     name="px")
                        for dc_ in range(DC):
                            nc.tensor.matmul(
                                px[:],
                                phT[dc_][:, 128 * t:128 * (t + 1)],
                                v16[:, D * dc_:D * (dc_ + 1)],
                                start=(dc_ == 0), stop=(dc_ == DC - 1))
                        res = sb.tile([128, D], F32, tag="h", bufs=16,
                                      name="res")
                        sA_t = (esum[:, rt:rt + 1] if sA is None
                                else sA[:, t:t + 1])
                        nc.vector.scalar_tensor_tensor(
                            res[:], h[rt][:], sA_t, px[:],
                            Alu.mult, Alu.add, accum_out=s1[:, rt:rt + 1])
                        scr = sb.tile([128, D], F32, tag="scr", bufs=2,
                                      name="scr")
                        nc.scalar.activation(scr[:], res[:], Act.Square,
                                             accum_out=s2[:, rt:rt + 1])
                        hres[rt] = res
                    recs_b = rec
                    if l == 0:
                        mu1, r1 = ln_stats(s1, s2, b, rk=recs_b,
                                           vsc=2048.0, rsc=45.254834)
                    else:
                        mu1, r1 = ln_stats(s1, s2, b, rk=recs_b)
                    rstd1[b] = r1
                    nm1 = sb.tile([128, 4], F32, tag="st4", bufs=16,
                                  name="nm1")
                    nc.vector.scalar_tensor_tensor(
                        nm1[:], mu1[:], -1.0, r1[:], Alu.mult, Alu.mult)
                    for t in range(4):
                        rt = 4 * b + t
                        tb = sb.tile([128, D], BF16, tag="h1bf", bufs=8,
                                     name=f"h1bf{rt}")
                        if t % 2 == 0:
                            nc.scalar.activation(tb[:], hres[rt][:],
                                                 Act.Identity,
                                                 bias=nm1[:, t:t + 1],
                                                 scale=r1[:, t:t + 1])
                        else:
                            nc.vector.tensor_scalar(
                                tb[:], hres[rt][:],
                                mu1[:, t:t + 1], r1[:, t:t + 1],
                                Alu.subtract, Alu.mult)
                        h1bf[rt] = tb

                # ---- feed-forward + residual + LN2 ----
                h_next = [None] * NT
                hbf_next = [None] * NT
                for b in range(BPC):
                    if ff8:
                        h1T8 = transpose_fp8(h1bf, b, f"g{l}T")
                        relu8 = [sb.tile([128, 2, S], FP8, tag="relu",
                                         bufs=16, name="relu8")
                                 for _ in range(FP)]
                        for f in range(FC):
                            pf = ps.tile([128, S], F32, tag="mm", bufs=3,
                                         name="pf")
                            fsl = slice(128 * f, 128 * (f + 1))
                            for p in range(DP):
                                nc.tensor.matmul(pf[:], w18[p][:, :, fsl],
                                                 h1T8[p][:],
                                                 start=(p == 0),
                                                 stop=(p == 1),
                                                 perf_mode=PM.DoubleRow)
                            dst = relu8[f // 2][:, f % 2, :]
                            if f % 2 == 0:
                                nc.scalar.activation(dst, pf[:], Act.Relu,
                                                     scale=SRELU / (SH * SW))
                            else:
                                nc.vector.tensor_scalar(
                                    dst, pf[:], SRELU / (SH * SW), 0.0,
                                    Alu.mult, Alu.max)
                        sF = sb.tile([128, 4], F32, tag="st4", bufs=16,
                                     name="sF")
                        nc.vector.tensor_scalar(sF[:], rstd1[b][:], KF, None,
                                                Alu.mult)
                    else:
                        h1T = transpose_bf(h1bf, b, f"g{l}T")
                        relu = []
                        for f in range(FC):
                            pf = ps.tile([128, S], F32, tag="mm", bufs=3,
                                         name="pf")
                            for din in range(DC):
                                nc.tensor.matmul(
                                    pf[:],
                                    w1[:, FF * din + 128 * f:
                                       FF * din + 128 * (f + 1)],
                                    h1T[din][:],
                                    start=(din == 0), stop=(din == DC - 1))
                            tr_ = sb.tile([128, S], BF16, tag="relu", bufs=16,
                                          name="relu")
                            if f % 2 == 0:
                                nc.scalar.activation(tr_[:], pf[:], Act.Relu)
                            else:
                                nc.vector.tensor_relu(tr_[:], pf[:])
                            relu.append(tr_)
                    for t in range(4):
                        rt = 4 * b + t
                        pd = ps.tile([128, D], F32, tag="mm", bufs=3,
                                     name="pd")
                        if ff8:
                            for p in range(FP):
                                nc.tensor.matmul(
                                    pd[:],
                                    relu8[p][:, :, 128 * t:128 * (t + 1)],
                                    w28[p][:],
                                    start=(p == 0), stop=(p == FP - 1),
                                    perf_mode=PM.DoubleRow)
                            sF_t = sF[:, t:t + 1]
                        else:
                            for fc in range(FC):
                                nc.tensor.matmul(
                                    pd[:],
                                    relu[fc][:, 128 * t:128 * (t + 1)],
                                    w2[:, D * fc:D * (fc + 1)],
                                    start=(fc == 0), stop=(fc == FC - 1))
                            sF_t = rstd1[b][:, t:t + 1]
                        res2 = sb.tile([128, D], F32, tag="h", bufs=16,
                                       name="res2")
                        nc.vector.scalar_tensor_tensor(
                            res2[:], hres[rt][:], sF_t, pd[:],
                            Alu.mult, Alu.add, accum_out=f1[:, rt:rt + 1])
                        scr2 = sb.tile([128, D], F32, tag="scr", bufs=2,
                                       name="scr2")
                        nc.scalar.activation(scr2[:], res2[:], Act.Square,
                                             accum_out=f2[:, rt:rt + 1])
                        h_next[rt] = res2
                    if ff8:
                        mu2, r2 = ln_stats(f1, f2, b, vsc=1.0 / (KF * KF),
                                           rsc=1.0 / KF)
                    else:
                        mu2, r2 = ln_stats(f1, f2, b)
                    rstd2[b] = r2
                    nm2 = sb.tile([128, 4], F32, tag="st4", bufs=16,
                                  name="nm2")
                    nc.vector.scalar_tensor_tensor(
                        nm2[:], mu2[:], -1.0, r2[:], Alu.mult, Alu.mult)
                    for t in range(4):
                        rt = 4 * b + t
                        tb = sb.tile([128, D], BF16, tag="hbf", bufs=10,
                                     name=f"nbf{rt}")
                        if t % 2 == 0:
                            nc.scalar.activation(tb[:], h_next[rt][:],
                                                 Act.Identity,
                                                 bias=nm2[:, t:t + 1],
                                                 scale=r2[:, t:t + 1])
                        else:
                            nc.vector.tensor_scalar(tb[:], h_next[rt][:],
                                                    mu2[:, t:t + 1],
                                                    r2[:, t:t + 1],
                                                    Alu.subtract, Alu.mult)
                        hbf_next[rt] = tb

                h = h_next
                hbf = hbf_next

            for rt in range(NT):
                dma(out_d[128 * rt:128 * (rt + 1), :], hbf[rt][:])

    nc.compile()
    return nc


def _host_inputs(inputs):
    x = np.asarray(inputs["x"])
    tok_emb = np.asarray(inputs["tok_emb"], dtype=np.float32)

    for nm in ("bq", "bk", "bv", "bo", "b1", "b2", "ln1_b", "ln2_b"):
        assert np.allclose(np.asarray(inputs[nm]), 0.0), f"{nm} nonzero"
    for nm in ("ln1_g", "ln2_g"):
        assert np.allclose(np.asarray(inputs[nm]), 1.0), f"{nm} != 1"

    f8 = ml_dtypes.float8_e4m3fn
    bf = ml_dtypes.bfloat16

    wq = np.asarray(inputs["Wq"], np.float64)
    wk = np.asarray(inputs["Wk"], np.float64)
    wv = np.asarray(inputs["Wv"], np.float64)
    wo = np.asarray(inputs["Wo"], np.float64)
    m = np.einsum('lde,lfe->ldf', wq, wk).astype(np.float32)  # Wq @ Wk^T
    v = np.einsum('lde,lef->ldf', wv, wo).astype(np.float32)  # Wv @ Wo

    m = m * SM
    assert np.abs(m).max() < 240.0, "fp8 overflow in fused M"

    def pairs8(w, scale):
        # [L, K, n] -> [L, K/256, 128, 2, n] with row k = 128*(2p+e)+r
        w = np.asarray(w, np.float32) * scale
        assert np.abs(w).max() < 240.0, "fp8 overflow"
        Lx, K, n = w.shape
        w = w.reshape(Lx, K // 256, 2, 128, n).transpose(0, 1, 3, 2, 4)
        return np.ascontiguousarray(w).astype(f8)

    # pair layout [L, DP, 128, 2, D]: row k = 128*(2p+e)+r
    m = m.reshape(L, DP, 2, 128, D).transpose(0, 1, 3, 2, 4)
    shared = {
        "m8": np.ascontiguousarray(m).astype(f8),
        "v16": v.astype(bf),
        "w1": np.asarray(inputs["W1"], np.float32).astype(bf),
        "w2": np.asarray(inputs["W2"], np.float32).astype(bf),
        "w18": pairs8(inputs["W1"], SW),
        "w28": pairs8(inputs["W2"], SW),
    }
    # transposed causal mask, diagonal block then zero padding:
    # col 0..127 = (0 if i >= j else -1e9), cols 128.. = 0
    jj = np.arange(128)
    cmT = np.zeros((128, S), dtype=np.float32)
    cmT[:, :128] = np.where(jj[None, :] >= jj[:, None], 0.0, -1e9)
    shared["cmT"] = cmT.astype(bf)

    h0 = tok_emb[x.astype(np.int64)]  # [B, S, D] fp32
    return shared, h0


def kernel(**inputs):
    global LAST_EXEC_NS
    shared, h0 = _host_inputs(inputs)

    if "prog" not in _CACHE:
        _CACHE["prog"] = _build_program()
    nc = _CACHE["prog"]

    in_maps = []
    for c in range(NCORES):
        m = dict(shared)
        m["h0"] = np.ascontiguousarray(
            h0[BPC * c:BPC * (c + 1)].reshape(R, D))
        m["h0b"] = m["h0"].astype(ml_dtypes.bfloat16)
        in_maps.append(m)

    trace = bool(int(os.environ.get("KERNEL_TRACE", "0")))
    res = bass_utils.run_bass_kernel_spmd(
        nc, in_maps, core_ids=list(range(NCORES)), trace=trace)
    LAST_EXEC_NS = res.exec_time_ns

    out = np.concatenate(
        [res.results[c]["out"].reshape(BPC, S, D) for c in range(NCORES)],
        axis=0)
    return out.astype(np.float32)
